# revision 1
# baseline (speedup 1.0000x reference)
"""Trainium2 Bass kernel for a LLaMA-style causal attention block.

Sharding (8 NeuronCores, one trn2 chip):
  - Tensor-parallel over heads: core c owns heads [4c, 4c+4) -> wq/wk/wv column
    slices [4096, 512]; computes qT/kT/v + RoPE + causal attention for its heads.
  - attnT [512, 2048] (bf16) is AllGather'd (chunked over 4 sq quarters, so comm
    overlaps compute) -> each core computes out[:, 512c:512c+512] = attn @ wo_cols.
  - Host concatenates the 8 column slices.

Layout trick: everything is computed transposed ([head_dim, seq]) so that no
on-device transposes are needed anywhere:
  qT/kT = w_h.T @ xT      (xT host-pretransposed)
  scoresT[sk, sq] = kT_tile.T @ qT     (softmax denom on DVE/GpSimd, not PE)
  attnT[hd, sq] = v_tile.T @ expT      (expT is exactly the scoresT layout)
  out[sq, cols] = attnT_full_tile.T @ wo_tile
RoPE is applied in the transposed layout with a DVE stream_shuffle partition
pair-swap. exp() needs no max-subtraction: scores are O(1) by construction.

Compute dtype bf16 (f32 PSUM accumulation), I/O f32.
"""

import math
import os
import sys

for _p in ("/opt/trn_rl_repo",):
    if os.path.isdir(_p) and _p not in sys.path:
        sys.path.insert(0, _p)

import numpy as np
import ml_dtypes

N_CORES = 8
B, S, D, H = 1, 2048, 4096, 32
HD = D // H          # 128
HPC = H // N_CORES   # 4 heads per core
CW = D // N_CORES    # 512 columns per core
NK = D // 128        # 32 contraction tiles
SQT = 512            # sq tile width
NSQ = S // SQT       # 4
SCALE = 1.0 / math.sqrt(HD)

_CACHE = {}
LAST_RESULT = None   # test harness reads exec_time_ns from here


def _build():
    import concourse.mybir as mybir
    import concourse.tile as tile
    from concourse import bacc, bass_isa

    dt = mybir.dt
    f32, bf16 = dt.float32, dt.bfloat16

    nc = bacc.Bacc("TRN2", target_bir_lowering=False, debug=False,
                   num_devices=N_CORES)

    xT = nc.dram_tensor("xT", [D, S], bf16, kind="ExternalInput").ap()
    wq = nc.dram_tensor("wq", [D, CW], bf16, kind="ExternalInput").ap()
    wk = nc.dram_tensor("wk", [D, CW], bf16, kind="ExternalInput").ap()
    wv = nc.dram_tensor("wv", [D, CW], bf16, kind="ExternalInput").ap()
    wo = nc.dram_tensor("wo", [D, CW], bf16, kind="ExternalInput").ap()
    cosT = nc.dram_tensor("cosT", [HD, S], bf16, kind="ExternalInput").ap()
    sinT = nc.dram_tensor("sinT", [HD, S], bf16, kind="ExternalInput").ap()
    ones = nc.dram_tensor("ones", [HD, 1], bf16, kind="ExternalInput").ap()
    masks = nc.dram_tensor("masks", [4, 128, SQT], bf16, kind="ExternalInput").ap()
    out = nc.dram_tensor("out", [S, CW], f32, kind="ExternalOutput").ap()

    swap_mask = []
    for i in range(16):
        swap_mask += [2 * i + 1, 2 * i]

    rg = [list(range(N_CORES))]

    with tile.TileContext(nc) as tc:
        with (
            tc.tile_pool(name="consts", bufs=1) as cpool,
            tc.tile_pool(name="xp", bufs=34) as xpool,
            tc.tile_pool(name="wqp", bufs=6) as wqp,
            tc.tile_pool(name="wkp", bufs=6) as wkp,
            tc.tile_pool(name="wvp", bufs=8) as wvp,
            tc.tile_pool(name="res", bufs=1) as res,
            tc.tile_pool(name="rope32", bufs=5) as rope32,
            tc.tile_pool(name="ropebf", bufs=6) as ropebf,
            tc.tile_pool(name="expp", bufs=8) as expp,
            tc.tile_pool(name="nrm", bufs=2) as nrm,
            tc.tile_pool(name="attnsb", bufs=4) as attnsb,
            tc.tile_pool(name="wop", bufs=1) as wop,
            tc.tile_pool(name="agsb", bufs=8) as agsb,
            tc.tile_pool(name="osb", bufs=5) as osb,
            tc.tile_pool(name="ps", bufs=8, space="PSUM") as ps,
            tc.tile_pool(name="dram", bufs=1, space="DRAM") as dram,
        ):
            # resident results of QKV+rope
            qrot = [res.tile([HD, S], bf16, name=f"qrot{h}") for h in range(HPC)]
            krot = [res.tile([HD, S], bf16, name=f"krot{h}") for h in range(HPC)]
            v_sb = [res.tile([128, CW], bf16, name=f"v{i}") for i in range(S // 128)]

            # AllGather bounce buffers (one per sq quarter)
            ag_in = [dram.tile([HPC * HD, SQT], bf16, name=f"agin{q}")
                     for q in range(NSQ)]
            ag_out = [dram.tile([D, SQT], bf16, addr_space="Shared",
                                name=f"agout{q}") for q in range(NSQ)]

            cos_sb = cpool.tile([HD, S], bf16, name="cos_sb")
            ones_sb = cpool.tile([HD, 1], bf16, name="ones_sb")
            sin_sb = cpool.tile([HD, S], bf16, name="sin_sb")
            mask_sb = [cpool.tile([128, SQT], bf16, name=f"mask{r}")
                       for r in range(4)]
            wo_sb = [wop.tile([128, CW], bf16, name=f"wo{d}") for d in range(NK)]

            def emit_qkv(st):
                sq0 = st * SQT
                q_ps = [ps.tile([128, SQT], f32, tag="b", name=f"qps{st}_{h}")
                        for h in range(HPC)]
                k_ps = [ps.tile([128, SQT], f32, tag="b", name=f"kps{st}_{h}")
                        for h in range(HPC)]
                x_tiles = []
                for d in range(NK):
                    xt = xpool.tile([128, SQT], bf16, tag="x", name=f"x{st}_{d}")
                    nc.sync.dma_start(xt[:], xT[d * 128:(d + 1) * 128,
                                                sq0:sq0 + SQT])
                    x_tiles.append(xt)
                    wqt = wqp.tile([128, CW], bf16, tag="wq", name=f"wq{st}_{d}")
                    nc.sync.dma_start(wqt[:], wq[d * 128:(d + 1) * 128, :])
                    wkt = wkp.tile([128, CW], bf16, tag="wk", name=f"wk{st}_{d}")
                    nc.sync.dma_start(wkt[:], wk[d * 128:(d + 1) * 128, :])
                    first, last = d == 0, d == NK - 1
                    for h in range(HPC):
                        nc.tensor.matmul(q_ps[h][:], wqt[:, h * HD:(h + 1) * HD],
                                         xt[:], start=first, stop=last)
                    for h in range(HPC):
                        nc.tensor.matmul(k_ps[h][:], wkt[:, h * HD:(h + 1) * HD],
                                         xt[:], start=first, stop=last)
                if st == 0:
                    # constants are first needed by RoPE / attention below;
                    # emitting them here keeps the first QKV DMAs in front
                    nc.sync.dma_start(cos_sb[:], cosT[:])
                    nc.sync.dma_start(sin_sb[:], sinT[:])
                    nc.sync.dma_start(ones_sb[:], ones[:])
                    for r in range(4):
                        nc.sync.dma_start(mask_sb[r][:], masks[r])
                # RoPE: rot = t*cos + shuffle(t)*sin'   (sin' sign-baked)
                for h in range(HPC):
                    for pst, rot in ((q_ps[h], qrot[h]), (k_ps[h], krot[h])):
                        tbf = ropebf.tile([128, SQT], bf16, tag="rbf",
                                          name=f"rbf{st}_{h}")
                        nc.scalar.copy(tbf[:], pst[:])
                        tsw = ropebf.tile([128, SQT], bf16, tag="rsw",
                                          name=f"rsw{st}_{h}")
                        nc.vector.stream_shuffle(tsw[:], tbf[:], swap_mask)
                        t1 = rope32.tile([128, SQT], f32, tag="r32",
                                         name=f"r1_{st}_{h}")
                        nc.vector.tensor_mul(t1[:], tbf[:],
                                             cos_sb[:, sq0:sq0 + SQT])
                        t2 = rope32.tile([128, SQT], f32, tag="r32",
                                         name=f"r2_{st}_{h}")
                        nc.vector.tensor_mul(t2[:], tsw[:],
                                             sin_sb[:, sq0:sq0 + SQT])
                        nc.vector.tensor_add(rot[:, sq0:sq0 + SQT], t1[:], t2[:])
                # V projection for this s range; all wv loads are issued
                # up front so the first V matmuls never wait on DMA
                wv_tiles = []
                for d in range(NK):
                    wvt = wvp.tile([128, CW], bf16, tag="wv", name=f"wv{st}_{d}")
                    nc.sync.dma_start(wvt[:], wv[d * 128:(d + 1) * 128, :])
                    wv_tiles.append(wvt)
                v_ps = [ps.tile([128, CW], f32, tag="b", name=f"vps{st}_{ss}")
                        for ss in range(4)]
                for d in range(NK):
                    first, last = d == 0, d == NK - 1
                    for ss in range(4):
                        nc.tensor.matmul(v_ps[ss][:],
                                         x_tiles[d][:, ss * 128:(ss + 1) * 128],
                                         wv_tiles[d][:], start=first, stop=last)
                for ss in range(4):
                    nc.scalar.copy(v_sb[st * 4 + ss][:], v_ps[ss][:])

            def emit_attention(sqT):
                sq0 = sqT * SQT
                nblk = 4 * (sqT + 1)
                a_tiles = []
                for h in range(HPC):
                    attn_ps = ps.tile([HD, SQT], f32, tag="b",
                                      name=f"aps{sqT}_{h}")
                    den_ps = ps.tile([1, SQT], f32, tag="b",
                                     name=f"dps{sqT}_{h}")
                    exp_tiles = []

                    def emit_pv(j, h=h, attn_ps=attn_ps, den_ps=den_ps,
                                exp_tiles=exp_tiles, nblk=nblk, sqT=sqT):
                        first, last = j == 0, j == nblk - 1
                        e, off = exp_tiles[j]
                        n = SQT - off
                        nc.tensor.matmul(attn_ps[:, off:SQT],
                                         v_sb[j][:, h * HD:(h + 1) * HD],
                                         e[:, 0:n],
                                         start=first, stop=last)
                        nc.tensor.matmul(den_ps[:, off:SQT], ones_sb[:],
                                         e[:, 0:n],
                                         start=first, stop=last)

                    for i in range(nblk):
                        r = i - 4 * sqT
                        # diagonal blocks: only sq >= sk is valid; skip the
                        # fully-masked leading columns entirely
                        off = max(0, r) * 128
                        n = SQT - off
                        sc = ps.tile([128, SQT], f32, tag="b",
                                     name=f"sc{sqT}_{h}_{i}")
                        nc.tensor.matmul(sc[:, 0:n],
                                         krot[h][:, i * 128:(i + 1) * 128],
                                         qrot[h][:, sq0 + off:sq0 + SQT],
                                         start=True, stop=True)
                        if r >= 0:  # triangular part within the first strip
                            nc.vector.tensor_add(sc[:, 0:n], sc[:, 0:n],
                                                 mask_sb[r][:, off:SQT])
                        e = expp.tile([128, SQT], bf16, tag="e",
                                      name=f"e{sqT}_{h}_{i}")
                        nc.scalar.activation(e[:, 0:n], sc[:, 0:n],
                                             mybir.ActivationFunctionType.Exp,
                                             scale=SCALE)
                        exp_tiles.append((e, off))
                        if i >= 2:
                            emit_pv(i - 2)
                    emit_pv(nblk - 2)
                    emit_pv(nblk - 1)

                    # evacuate PSUM right away so the next round's projections
                    # get their banks back without waiting on the normalize
                    rec = nrm.tile([1, SQT], f32, tag="rec",
                                   name=f"rec{sqT}_{h}")
                    nc.vector.reciprocal(rec[:], den_ps[:])
                    bc = nrm.tile([128, SQT], f32, tag="bc",
                                  name=f"bc{sqT}_{h}")
                    nc.gpsimd.partition_broadcast(bc[:], rec[:], channels=128)
                    a_sb = attnsb.tile([HD, SQT], bf16, tag="a",
                                       name=f"asb{sqT}_{h}")
                    nc.vector.tensor_mul(a_sb[:], attn_ps[:], bc[:])
                    a_tiles.append(a_sb)
                return a_tiles

            def emit_attn_stores(sqT, a_tiles):
                # stores are emitted one round late so they never sit at the
                # head of the in-order DMA queue blocking the next round's
                # ready-to-issue loads
                for h in range(HPC):
                    nc.sync.dma_start(ag_in[sqT][h * HD:(h + 1) * HD, :],
                                      a_tiles[h][:])
                nc.gpsimd.collective_compute(
                    "AllGather", mybir.AluOpType.bypass, replica_groups=rg,
                    ins=[ag_in[sqT].opt()], outs=[ag_out[sqT].opt()])

            # attention for round st is emitted after QKV round st+1 (its
            # matmuls fill PSUM-release stalls at QKV round boundaries); its
            # stores go out one round later still
            emit_qkv(0)
            pending = None
            for st in range(1, NSQ):
                emit_qkv(st)
                if pending is not None:
                    emit_attn_stores(st - 2, pending)
                pending = emit_attention(st - 1)
                if st == 1:
                    for d in range(NK):  # prefetch wo during attention
                        nc.sync.dma_start(wo_sb[d][:],
                                          wo[d * 128:(d + 1) * 128, :])
            emit_attn_stores(NSQ - 2, pending)
            pending = emit_attention(NSQ - 1)
            emit_attn_stores(NSQ - 1, pending)

            # ================= output projection =================
            pending_o = None
            for q in range(NSQ):
                o_ps = [ps.tile([128, CW], f32, tag="b", name=f"ops{q}_{ss}")
                        for ss in range(4)]
                for d in range(NK):
                    agt = agsb.tile([128, SQT], bf16, tag="ag",
                                    name=f"agt{q}_{d}")
                    nc.sync.dma_start(agt[:],
                                      ag_out[q][d * 128:(d + 1) * 128, :])
                    first, last = d == 0, d == NK - 1
                    for ss in range(4):
                        nc.tensor.matmul(o_ps[ss][:],
                                         agt[:, ss * 128:(ss + 1) * 128],
                                         wo_sb[d][:], start=first, stop=last)
                    if d == 4 and pending_o is not None:
                        # previous quarter's stores, emitted after this
                        # quarter's first loads (no DMA-queue blocking)
                        qq, tiles = pending_o
                        for ss in range(4):
                            nc.sync.dma_start(
                                out[qq * SQT + ss * 128:
                                    qq * SQT + (ss + 1) * 128, :],
                                tiles[ss][:])
                        pending_o = None
                o_tiles = []
                for ss in range(4):
                    o = osb.tile([128, CW], f32, tag="o", name=f"o{q}_{ss}")
                    nc.scalar.copy(o[:], o_ps[ss][:])
                    o_tiles.append(o)
                pending_o = (q, o_tiles)
            qq, tiles = pending_o
            for ss in range(4):
                nc.sync.dma_start(
                    out[qq * SQT + ss * 128:qq * SQT + (ss + 1) * 128, :],
                    tiles[ss][:])

    nc.compile()
    return nc


def _prep_inputs(x, wq, wk, wv, wo, freqs_cos, freqs_sin, mask):
    bf16 = ml_dtypes.bfloat16
    x2 = np.asarray(x, dtype=np.float32).reshape(S, D)
    xT = np.ascontiguousarray(x2.T).astype(bf16)
    cosT = np.repeat(np.asarray(freqs_cos, np.float32).T, 2, axis=0)
    sinT = np.repeat(np.asarray(freqs_sin, np.float32).T, 2, axis=0).copy()
    sinT[0::2] *= -1.0
    cosT = np.ascontiguousarray(cosT).astype(bf16)
    sinT = np.ascontiguousarray(sinT).astype(bf16)
    m2 = np.asarray(mask, np.float32).reshape(S, S)
    masks = np.stack([np.ascontiguousarray(m2[0:SQT, r * 128:(r + 1) * 128].T)
                      for r in range(4)]).astype(bf16)  # [4, 128, 512]
    in_maps = []
    for c in range(N_CORES):
        cols = slice(c * CW, (c + 1) * CW)
        in_maps.append({
            "xT": xT,
            "wq": np.ascontiguousarray(np.asarray(wq, np.float32)[:, cols]).astype(bf16),
            "wk": np.ascontiguousarray(np.asarray(wk, np.float32)[:, cols]).astype(bf16),
            "wv": np.ascontiguousarray(np.asarray(wv, np.float32)[:, cols]).astype(bf16),
            "wo": np.ascontiguousarray(np.asarray(wo, np.float32)[:, cols]).astype(bf16),
            "cosT": cosT,
            "ones": np.ones((HD, 1), bf16),
            "sinT": sinT,
            "masks": masks,
        })
    return in_maps


def kernel(x, wq, wk, wv, wo, freqs_cos, freqs_sin, mask):
    global LAST_RESULT
    from concourse.bass_utils import run_bass_kernel_spmd

    if "nc" not in _CACHE:
        _CACHE["nc"] = _build()
    nc = _CACHE["nc"]
    in_maps = _prep_inputs(x, wq, wk, wv, wo, freqs_cos, freqs_sin, mask)
    res = run_bass_kernel_spmd(nc, in_maps, core_ids=list(range(N_CORES)))
    LAST_RESULT = res
    out = np.concatenate([res.results[c]["out"] for c in range(N_CORES)],
                         axis=1)
    return out.reshape(B, S, D).astype(np.float32)



# revision 10
# speedup vs baseline: 1.0797x; 1.0797x over previous
"""Trainium2 Bass kernel for a LLaMA-style causal attention block.

Sharding (8 NeuronCores, one trn2 chip):
  - Tensor-parallel over heads: core c owns heads [4c, 4c+4) -> wq/wk/wv column
    slices [4096, 512]; computes qT/kT/v + RoPE + causal attention for its heads.
  - attnT [512, 2048] (bf16) is AllGather'd per sq quarter -> each core computes
    out[:, 512c:512c+512] = attn @ wo_cols.  Host concatenates column slices.

Layout trick: everything is computed transposed ([head_dim, seq]) so no
on-device transposes are needed:
  qT/kT = w_h.T @ xT      (xT host-pretransposed)
  scoresT[sk, sq] = kT_tile.T @ qT
  attnT[hd, sq] = v_tile.T @ expT
  out[sq, cols] = attnT_full_tile.T @ wo_tile
exp() needs no max-subtraction: scores are O(1) by construction.

v2 structure (vs baseline):
  - wq/wk/wv resident in SBUF, loaded ONCE on three parallel DMA queues
    (baseline re-streamed 12.6MB of weights per strip -> QKV DMA starvation,
    PE idle gaps -> HAM K=4/8 re-throttles).
  - Each strip = three 4-bank passes: A={q(h0),q(h1),k(h0),k(h1)},
    B={q(h2),q(h3),k(h2),k(h3)}, C={v}; attention of the previous strip runs
    as a fourth pass.  At most 8 PSUM banks live, each bank has ~30us of
    evacuation slack -> PE never waits on PSUM.
  - softmax denominator: exp blocks accumulated on DVE into an f32 tile,
    then ONE ones-matmul per (head, quarter) (baseline: a [1,n] matmul per
    block = 69k wasted PE cycles + 160 LDWEIGHTS).
  - 1/den via reciprocal_approx_fast (5x faster than InstReciprocal) and the
    attn PSUM bank is held only until one DVE multiply after broadcast.
  - output stored bf16 (host upcasts), halving the store tail.

Compute dtype bf16 (f32 PSUM accumulation), I/O f32.
"""

import math
import os
import sys

for _p in ("/opt/trn_rl_repo",):
    if os.path.isdir(_p) and _p not in sys.path:
        sys.path.insert(0, _p)

import numpy as np
import ml_dtypes

N_CORES = 8
B, S, D, H = 1, 2048, 4096, 32
HD = D // H          # 128
HPC = H // N_CORES   # 4 heads per core
CW = D // N_CORES    # 512 columns per core
NK = D // 128        # 32 contraction tiles
SQT = 512            # sq tile width
NSQ = S // SQT       # 4
SCALE = 1.0 / math.sqrt(HD)

_CACHE = {}
LAST_RESULT = None   # test harness reads exec_time_ns from here


def _build():
    import concourse.mybir as mybir
    import concourse.tile as tile
    from concourse import bacc

    dt = mybir.dt
    f32, bf16 = dt.float32, dt.bfloat16

    nc = bacc.Bacc("TRN2", target_bir_lowering=False, debug=False,
                   num_devices=N_CORES)

    xT = nc.dram_tensor("xT", [D, S], bf16, kind="ExternalInput").ap()
    wq = nc.dram_tensor("wq", [D, CW], bf16, kind="ExternalInput").ap()
    wk = nc.dram_tensor("wk", [D, CW], bf16, kind="ExternalInput").ap()
    wv = nc.dram_tensor("wv", [D, CW], bf16, kind="ExternalInput").ap()
    wo = nc.dram_tensor("wo", [D, CW], bf16, kind="ExternalInput").ap()
    cosT = nc.dram_tensor("cosT", [HD, S], bf16, kind="ExternalInput").ap()
    sinT = nc.dram_tensor("sinT", [HD, S], bf16, kind="ExternalInput").ap()
    ones = nc.dram_tensor("ones", [HD, 1], bf16, kind="ExternalInput").ap()
    masks = nc.dram_tensor("masks", [4, 128, SQT], bf16, kind="ExternalInput").ap()
    out = nc.dram_tensor("out", [S, CW], bf16, kind="ExternalOutput").ap()

    swap_mask = []
    for i in range(16):
        swap_mask += [2 * i + 1, 2 * i]

    rg = [list(range(N_CORES))]

    with tile.TileContext(nc) as tc:
        with (
            tc.tile_pool(name="consts", bufs=1) as cpool,
            tc.tile_pool(name="wqp", bufs=NK) as wqp,    # wq resident; reused by wo
            tc.tile_pool(name="wkp", bufs=NK) as wkp,    # wk resident; reused by ag
            tc.tile_pool(name="wvp", bufs=NK) as wvp,    # wv resident
            tc.tile_pool(name="xp", bufs=33) as xpool,   # x strip ring
            tc.tile_pool(name="res", bufs=1) as res,     # qrot/krot/v_sb
            tc.tile_pool(name="rope", bufs=2) as ropep,
            tc.tile_pool(name="expp", bufs=4) as expp,
            tc.tile_pool(name="accp", bufs=1) as accp,
            tc.tile_pool(name="nrm", bufs=1) as nrm,
            tc.tile_pool(name="attnsb", bufs=2) as attnsb,
            tc.tile_pool(name="ps", bufs=8, space="PSUM") as ps,
            tc.tile_pool(name="dram", bufs=1, space="DRAM") as dram,
        ):
            # resident results of QKV+rope
            qrot = [res.tile([HD, S], bf16, name=f"qrot{h}") for h in range(HPC)]
            krot = [res.tile([HD, S], bf16, name=f"krot{h}") for h in range(HPC)]
            v_sb = [res.tile([128, CW], bf16, name=f"v{i}") for i in range(S // 128)]

            # AllGather bounce buffers (one per sq quarter)
            ag_in = [dram.tile([HPC * HD, SQT], bf16, name=f"agin{q}")
                     for q in range(NSQ)]
            ag_out = [dram.tile([D, SQT], bf16, addr_space="Shared",
                                name=f"agout{q}") for q in range(NSQ)]

            cos_sb = cpool.tile([HD, S], bf16, name="cos_sb")
            sin_sb = cpool.tile([HD, S], bf16, name="sin_sb")
            ones_sb = cpool.tile([HD, 1], bf16, name="ones_sb")
            mask_sb = [cpool.tile([128, SQT], bf16, name=f"mask{r}")
                       for r in range(4)]

            # ---- resident weights, loaded once on parallel queues ----
            # (only SP/Activation/gpsimd can issue DMAs; x streams on SP, so
            # wq rides the Activation queue and wk/consts ride gpsimd. wv is
            # issued on SP inside strip 0 after its x tiles — it is first
            # needed by strip 0's PASS C, ~66us in.)
            wq_sb, wk_sb, wv_sb = [], [], []
            for d in range(NK):
                wqt = wqp.tile([128, CW], bf16, tag="wq", name=f"wq{d}")
                nc.scalar.dma_start(wqt[:], wq[d * 128:(d + 1) * 128, :])
                wq_sb.append(wqt)
                wkt = wkp.tile([128, CW], bf16, tag="wk", name=f"wk{d}")
                nc.gpsimd.dma_start(wkt[:], wk[d * 128:(d + 1) * 128, :])
                wk_sb.append(wkt)
            nc.gpsimd.dma_start(cos_sb[:], cosT[:])
            nc.gpsimd.dma_start(sin_sb[:], sinT[:])
            nc.gpsimd.dma_start(ones_sb[:], ones[:])
            for r in range(4):
                nc.gpsimd.dma_start(mask_sb[r][:], masks[r])

            wo_sb = []   # filled during strip 3 (reuses wqp slots)

            def emit_rope(pst, rot, sq0):
                # rot = t*cos + shuffle(t)*sin'   (sin' sign-baked)
                tbf = ropep.tile([128, SQT], bf16, tag="rbf", name="rbf")
                nc.scalar.copy(tbf[:], pst[:])          # frees the PSUM bank
                tsw = ropep.tile([128, SQT], bf16, tag="rsw", name="rsw")
                nc.vector.stream_shuffle(tsw[:], tbf[:], swap_mask)
                nc.vector.tensor_mul(tbf[:], tbf[:], cos_sb[:, sq0:sq0 + SQT])
                nc.vector.tensor_mul(tsw[:], tsw[:], sin_sb[:, sq0:sq0 + SQT])
                nc.vector.tensor_add(rot[:, sq0:sq0 + SQT], tbf[:], tsw[:])

            def emit_strip(st):
                sq0 = st * SQT
                x_tiles = []
                for d in range(NK):
                    xt = xpool.tile([128, SQT], bf16, tag="x", name=f"x{st}_{d}")
                    nc.sync.dma_start(xt[:], xT[d * 128:(d + 1) * 128,
                                                sq0:sq0 + SQT])
                    x_tiles.append(xt)
                if st == 0:
                    for d in range(NK):
                        wvt = wvp.tile([128, CW], bf16, tag="wv", name=f"wv{d}")
                        nc.sync.dma_start(wvt[:], wv[d * 128:(d + 1) * 128, :])
                        wv_sb.append(wvt)
                # PASS A: q/k for heads 0,1   PASS B: q/k for heads 2,3
                for hp in range(2):
                    h0, h1 = 2 * hp, 2 * hp + 1
                    q0 = ps.tile([128, SQT], f32, tag="b", name=f"qps{st}_{h0}")
                    q1 = ps.tile([128, SQT], f32, tag="b", name=f"qps{st}_{h1}")
                    k0 = ps.tile([128, SQT], f32, tag="b", name=f"kps{st}_{h0}")
                    k1 = ps.tile([128, SQT], f32, tag="b", name=f"kps{st}_{h1}")
                    for d in range(NK):
                        first, last = d == 0, d == NK - 1
                        xt = x_tiles[d]
                        nc.tensor.matmul(q0[:], wq_sb[d][:, h0 * HD:(h0 + 1) * HD],
                                         xt[:], start=first, stop=last)
                        nc.tensor.matmul(q1[:], wq_sb[d][:, h1 * HD:(h1 + 1) * HD],
                                         xt[:], start=first, stop=last)
                        nc.tensor.matmul(k0[:], wk_sb[d][:, h0 * HD:(h0 + 1) * HD],
                                         xt[:], start=first, stop=last)
                        nc.tensor.matmul(k1[:], wk_sb[d][:, h1 * HD:(h1 + 1) * HD],
                                         xt[:], start=first, stop=last)
                    emit_rope(q0, qrot[h0], sq0)
                    emit_rope(q1, qrot[h1], sq0)
                    emit_rope(k0, krot[h0], sq0)
                    emit_rope(k1, krot[h1], sq0)
                    if st == 3 and hp == 0:
                        # wq dead after this strip's A/B passes; prefetch wo
                        # into the freed slots for the output projection
                        for d in range(NK):
                            wot = wqp.tile([128, CW], bf16, tag="wq",
                                           name=f"wo{d}")
                            nc.gpsimd.dma_start(wot[:],
                                                wo[d * 128:(d + 1) * 128, :])
                            wo_sb.append(wot)
                # PASS C: v projection
                v_ps = [ps.tile([128, CW], f32, tag="b", name=f"vps{st}_{ss}")
                        for ss in range(4)]
                for d in range(NK):
                    first, last = d == 0, d == NK - 1
                    for ss in range(4):
                        nc.tensor.matmul(v_ps[ss][:],
                                         x_tiles[d][:, ss * 128:(ss + 1) * 128],
                                         wv_sb[d][:], start=first, stop=last)
                for ss in range(4):
                    nc.scalar.copy(v_sb[st * 4 + ss][:], v_ps[ss][:])

            def emit_attention(sqT):
                sq0 = sqT * SQT
                nblk = 4 * (sqT + 1)
                for h in range(HPC):
                    attn_ps = ps.tile([HD, SQT], f32, tag="b",
                                      name=f"aps{sqT}_{h}")
                    acc = accp.tile([128, SQT], f32, tag="acc",
                                    name=f"acc{sqT}_{h}")
                    exp_tiles = []

                    def emit_pv(j, h=h, attn_ps=attn_ps,
                                exp_tiles=exp_tiles, nblk=nblk):
                        first, last = j == 0, j == nblk - 1
                        e, off = exp_tiles[j]
                        n = SQT - off
                        nc.tensor.matmul(attn_ps[:, off:SQT],
                                         v_sb[j][:, h * HD:(h + 1) * HD],
                                         e[:, 0:n],
                                         start=first, stop=last)

                    for i in range(nblk):
                        r = i - 4 * sqT
                        # diagonal blocks: only sq >= sk is valid; skip the
                        # fully-masked leading columns entirely
                        off = max(0, r) * 128
                        n = SQT - off
                        sc = ps.tile([128, SQT], f32, tag="b",
                                     name=f"sc{sqT}_{h}_{i}")
                        nc.tensor.matmul(sc[:, 0:n],
                                         krot[h][:, i * 128:(i + 1) * 128],
                                         qrot[h][:, sq0 + off:sq0 + SQT],
                                         start=True, stop=True)
                        if r >= 0:  # triangular part within the first strip
                            nc.vector.tensor_add(sc[:, 0:n], sc[:, 0:n],
                                                 mask_sb[r][:, off:SQT])
                        e = expp.tile([128, SQT], bf16, tag="e",
                                      name=f"e{sqT}_{h}_{i}")
                        nc.scalar.activation(e[:, 0:n], sc[:, 0:n],
                                             mybir.ActivationFunctionType.Exp,
                                             scale=SCALE)
                        # denominator: accumulate exp on DVE instead of a
                        # per-block [1,n] PE matmul
                        if i == 0:
                            nc.vector.tensor_copy(acc[:], e[:])
                        else:
                            nc.vector.tensor_add(acc[:, off:SQT],
                                                 acc[:, off:SQT], e[:, 0:n])
                        exp_tiles.append((e, off))
                        if i >= 2:
                            emit_pv(i - 2)
                    emit_pv(nblk - 2)
                    emit_pv(nblk - 1)

                    # acc stays f32 through the accumulation; one bf16
                    # rounding before the ones-reduce averages out over the
                    # 128-partition sum (~0.4%/sqrt(128) den error)
                    acc_bf = accp.tile([128, SQT], bf16, tag="accbf",
                                       bufs=1, name=f"accbf{sqT}_{h}")
                    nc.vector.tensor_copy(acc_bf[:], acc[:])
                    den_ps = ps.tile([1, SQT], f32, tag="b",
                                     name=f"dps{sqT}_{h}")
                    nc.tensor.matmul(den_ps[:], ones_sb[:], acc_bf[:],
                                     start=True, stop=True)
                    rec = nrm.tile([1, SQT], f32, tag="rec",
                                   name=f"rec{sqT}_{h}")
                    nc.vector.reciprocal_approx_fast(out=rec[:], in_=den_ps[:])
                    bc = nrm.tile([128, SQT], f32, tag="bc",
                                  name=f"bc{sqT}_{h}")
                    nc.gpsimd.partition_broadcast(bc[:], rec[:], channels=128)
                    a_sb = attnsb.tile([HD, SQT], bf16, tag="a",
                                       name=f"asb{sqT}_{h}")
                    nc.vector.tensor_mul(a_sb[:], attn_ps[:], bc[:])
                    # store on the gpsimd queue so the sync queue's x/ag loads
                    # are never blocked behind stores
                    nc.gpsimd.dma_start(ag_in[sqT][h * HD:(h + 1) * HD, :],
                                        a_sb[:])
                nc.gpsimd.collective_compute(
                    "AllGather", mybir.AluOpType.bypass, replica_groups=rg,
                    ins=[ag_in[sqT].opt()], outs=[ag_out[sqT].opt()])

            emit_strip(0)
            for st in range(1, NSQ):
                emit_strip(st)
                emit_attention(st - 1)
            emit_attention(NSQ - 1)

            # ================= output projection =================
            for q in range(NSQ):
                o_ps = [ps.tile([128, CW], f32, tag="b", name=f"ops{q}_{ss}")
                        for ss in range(4)]
                for d in range(NK):
                    agt = wkp.tile([128, SQT], bf16, tag="wk",
                                   name=f"agt{q}_{d}")
                    nc.sync.dma_start(agt[:],
                                      ag_out[q][d * 128:(d + 1) * 128, :])
                    first, last = d == 0, d == NK - 1
                    for ss in range(4):
                        nc.tensor.matmul(o_ps[ss][:],
                                         agt[:, ss * 128:(ss + 1) * 128],
                                         wo_sb[d][:], start=first, stop=last)
                for ss in range(4):
                    o = attnsb.tile([128, CW], bf16, tag="a", name=f"o{q}_{ss}")
                    nc.scalar.copy(o[:], o_ps[ss][:])
                    nc.gpsimd.dma_start(
                        out[q * SQT + ss * 128:q * SQT + (ss + 1) * 128, :],
                        o[:])

    nc.compile()
    return nc


def _prep_inputs(x, wq, wk, wv, wo, freqs_cos, freqs_sin, mask):
    bf16 = ml_dtypes.bfloat16
    x2 = np.asarray(x, dtype=np.float32).reshape(S, D)
    xT = np.ascontiguousarray(x2.T).astype(bf16)
    cosT = np.repeat(np.asarray(freqs_cos, np.float32).T, 2, axis=0)
    sinT = np.repeat(np.asarray(freqs_sin, np.float32).T, 2, axis=0).copy()
    sinT[0::2] *= -1.0
    cosT = np.ascontiguousarray(cosT).astype(bf16)
    sinT = np.ascontiguousarray(sinT).astype(bf16)
    m2 = np.asarray(mask, np.float32).reshape(S, S)
    masks = np.stack([np.ascontiguousarray(m2[0:SQT, r * 128:(r + 1) * 128].T)
                      for r in range(4)]).astype(bf16)  # [4, 128, 512]
    in_maps = []
    for c in range(N_CORES):
        cols = slice(c * CW, (c + 1) * CW)
        in_maps.append({
            "xT": xT,
            "wq": np.ascontiguousarray(np.asarray(wq, np.float32)[:, cols]).astype(bf16),
            "wk": np.ascontiguousarray(np.asarray(wk, np.float32)[:, cols]).astype(bf16),
            "wv": np.ascontiguousarray(np.asarray(wv, np.float32)[:, cols]).astype(bf16),
            "wo": np.ascontiguousarray(np.asarray(wo, np.float32)[:, cols]).astype(bf16),
            "cosT": cosT,
            "ones": np.ones((HD, 1), bf16),
            "sinT": sinT,
            "masks": masks,
        })
    return in_maps


def kernel(x, wq, wk, wv, wo, freqs_cos, freqs_sin, mask):
    global LAST_RESULT
    from concourse.bass_utils import run_bass_kernel_spmd

    if "nc" not in _CACHE:
        _CACHE["nc"] = _build()
    nc = _CACHE["nc"]
    in_maps = _prep_inputs(x, wq, wk, wv, wo, freqs_cos, freqs_sin, mask)
    res = run_bass_kernel_spmd(nc, in_maps, core_ids=list(range(N_CORES)))
    LAST_RESULT = res
    out = np.concatenate([res.results[c]["out"].astype(np.float32)
                          for c in range(N_CORES)], axis=1)
    return out.reshape(B, S, D)


# revision 12
# speedup vs baseline: 1.1053x; 1.0238x over previous
"""Trainium2 Bass kernel for a LLaMA-style causal attention block.

Sharding (8 NeuronCores, one trn2 chip):
  - Tensor-parallel over heads: core c owns heads [4c, 4c+4) -> wq/wk/wv column
    slices [4096, 512]; computes qT/kT/v + RoPE + causal attention for its heads.
  - attnT [512, 2048] (bf16) is AllGather'd per sq quarter -> each core computes
    out[:, 512c:512c+512] = attn @ wo_cols.  Host concatenates column slices.

Layout trick: everything is computed transposed ([head_dim, seq]) so no
on-device transposes are needed:
  qT/kT = w_h.T @ xT      (xT host-pretransposed)
  scoresT[sk, sq] = kT_tile.T @ qT
  attnT[hd, sq] = v_tile.T @ expT
  out[sq, cols] = attnT_full_tile.T @ wo_tile
exp() needs no max-subtraction: scores are O(1) by construction.

v2 structure (vs baseline):
  - wq/wk/wv resident in SBUF, loaded ONCE on three parallel DMA queues
    (baseline re-streamed 12.6MB of weights per strip -> QKV DMA starvation,
    PE idle gaps -> HAM K=4/8 re-throttles).
  - Each strip = three 4-bank passes: A={q(h0),q(h1),k(h0),k(h1)},
    B={q(h2),q(h3),k(h2),k(h3)}, C={v}; attention of the previous strip runs
    as a fourth pass.  At most 8 PSUM banks live, each bank has ~30us of
    evacuation slack -> PE never waits on PSUM.
  - softmax denominator: exp blocks accumulated on DVE into an f32 tile,
    then ONE ones-matmul per (head, quarter) (baseline: a [1,n] matmul per
    block = 69k wasted PE cycles + 160 LDWEIGHTS).
  - 1/den via reciprocal_approx_fast (5x faster than InstReciprocal) and the
    attn PSUM bank is held only until one DVE multiply after broadcast.
  - output stored bf16 (host upcasts), halving the store tail.

Compute dtype bf16 (f32 PSUM accumulation), I/O f32.
"""

import math
import os
import sys

for _p in ("/opt/trn_rl_repo",):
    if os.path.isdir(_p) and _p not in sys.path:
        sys.path.insert(0, _p)

import numpy as np
import ml_dtypes

N_CORES = 8
B, S, D, H = 1, 2048, 4096, 32
HD = D // H          # 128
HPC = H // N_CORES   # 4 heads per core
CW = D // N_CORES    # 512 columns per core
NK = D // 128        # 32 contraction tiles
SQT = 512            # sq tile width
NSQ = S // SQT       # 4
SCALE = 1.0 / math.sqrt(HD)

_CACHE = {}
LAST_RESULT = None   # test harness reads exec_time_ns from here


def _build():
    import concourse.mybir as mybir
    import concourse.tile as tile
    from concourse import bacc

    dt = mybir.dt
    f32, bf16 = dt.float32, dt.bfloat16

    nc = bacc.Bacc("TRN2", target_bir_lowering=False, debug=False,
                   num_devices=N_CORES)

    xT = nc.dram_tensor("xT", [D, S], bf16, kind="ExternalInput").ap()
    wq = nc.dram_tensor("wq", [D, CW], bf16, kind="ExternalInput").ap()
    wk = nc.dram_tensor("wk", [D, CW], bf16, kind="ExternalInput").ap()
    wv = nc.dram_tensor("wv", [D, CW], bf16, kind="ExternalInput").ap()
    wo = nc.dram_tensor("wo", [D, CW], bf16, kind="ExternalInput").ap()
    cosT = nc.dram_tensor("cosT", [HD, S], bf16, kind="ExternalInput").ap()
    sinT = nc.dram_tensor("sinT", [HD, S], bf16, kind="ExternalInput").ap()
    ones = nc.dram_tensor("ones", [HD, 1], bf16, kind="ExternalInput").ap()
    masks = nc.dram_tensor("masks", [4, 128, SQT], bf16, kind="ExternalInput").ap()
    out = nc.dram_tensor("out", [S, CW], bf16, kind="ExternalOutput").ap()

    swap_mask = []
    for i in range(16):
        swap_mask += [2 * i + 1, 2 * i]

    rg = [list(range(N_CORES))]

    with tile.TileContext(nc) as tc:
        with (
            tc.tile_pool(name="consts", bufs=1) as cpool,
            tc.tile_pool(name="wqp", bufs=NK) as wqp,    # wq resident; reused by wo
            tc.tile_pool(name="wkp", bufs=NK) as wkp,    # wk resident; reused by ag
            tc.tile_pool(name="wvp", bufs=NK) as wvp,    # wv resident
            tc.tile_pool(name="xp", bufs=33) as xpool,   # x strip ring
            tc.tile_pool(name="res", bufs=1) as res,     # qrot/krot/v_sb
            tc.tile_pool(name="rope", bufs=2) as ropep,
            tc.tile_pool(name="expp", bufs=4) as expp,
            tc.tile_pool(name="accp", bufs=1) as accp,
            tc.tile_pool(name="nrm", bufs=1) as nrm,
            tc.tile_pool(name="attnsb", bufs=2) as attnsb,
            tc.tile_pool(name="ps", bufs=8, space="PSUM") as ps,
            tc.tile_pool(name="dram", bufs=1, space="DRAM") as dram,
        ):
            # resident results of QKV+rope
            qrot = [res.tile([HD, S], bf16, name=f"qrot{h}") for h in range(HPC)]
            krot = [res.tile([HD, S], bf16, name=f"krot{h}") for h in range(HPC)]
            v_sb = [res.tile([128, CW], bf16, name=f"v{i}") for i in range(S // 128)]

            # AllGather bounce buffers (one per sq quarter)
            ag_in = [dram.tile([HPC * HD, SQT], bf16, name=f"agin{q}")
                     for q in range(NSQ)]
            ag_out = [dram.tile([D, SQT], bf16, addr_space="Shared",
                                name=f"agout{q}") for q in range(NSQ)]

            cos_sb = cpool.tile([HD, S], bf16, name="cos_sb")
            sin_sb = cpool.tile([HD, S], bf16, name="sin_sb")
            ones_sb = cpool.tile([HD, 1], bf16, name="ones_sb")
            mask_sb = [cpool.tile([128, SQT], bf16, name=f"mask{r}")
                       for r in range(4)]

            # ---- resident weights, loaded once on parallel queues ----
            # (only SP/Activation/gpsimd can issue DMAs; x streams on SP, so
            # wq rides the Activation queue and wk/consts ride gpsimd. wv is
            # issued on SP inside strip 0 after its x tiles — it is first
            # needed by strip 0's PASS C, ~66us in.)
            wq_sb, wk_sb, wv_sb = [], [], []
            for d in range(NK):
                wqt = wqp.tile([128, CW], bf16, tag="wq", name=f"wq{d}")
                nc.scalar.dma_start(wqt[:], wq[d * 128:(d + 1) * 128, :])
                wq_sb.append(wqt)
                wkt = wkp.tile([128, CW], bf16, tag="wk", name=f"wk{d}")
                nc.gpsimd.dma_start(wkt[:], wk[d * 128:(d + 1) * 128, :])
                wk_sb.append(wkt)
            nc.gpsimd.dma_start(cos_sb[:], cosT[:])
            nc.gpsimd.dma_start(sin_sb[:], sinT[:])
            nc.gpsimd.dma_start(ones_sb[:], ones[:])
            for r in range(4):
                nc.gpsimd.dma_start(mask_sb[r][:], masks[r])

            wo_sb = []   # filled during strip 3 (reuses wqp slots)

            def emit_rope(pst, rot, sq0):
                # rot = t*cos + shuffle(t)*sin'   (sin' sign-baked)
                tbf = ropep.tile([128, SQT], bf16, tag="rbf", name="rbf")
                nc.scalar.copy(tbf[:], pst[:])          # frees the PSUM bank
                tsw = ropep.tile([128, SQT], bf16, tag="rsw", name="rsw")
                nc.vector.stream_shuffle(tsw[:], tbf[:], swap_mask)
                nc.vector.tensor_mul(tbf[:], tbf[:], cos_sb[:, sq0:sq0 + SQT])
                nc.vector.tensor_mul(tsw[:], tsw[:], sin_sb[:, sq0:sq0 + SQT])
                nc.vector.tensor_add(rot[:, sq0:sq0 + SQT], tbf[:], tsw[:])

            def emit_strip(st):
                sq0 = st * SQT
                x_tiles = []
                for d in range(NK):
                    xt = xpool.tile([128, SQT], bf16, tag="x", name=f"x{st}_{d}")
                    nc.sync.dma_start(xt[:], xT[d * 128:(d + 1) * 128,
                                                sq0:sq0 + SQT])
                    x_tiles.append(xt)
                if st == 0:
                    for d in range(NK):
                        wvt = wvp.tile([128, CW], bf16, tag="wv", name=f"wv{d}")
                        nc.sync.dma_start(wvt[:], wv[d * 128:(d + 1) * 128, :])
                        wv_sb.append(wvt)
                # PASS A: q heads 0-3 (streams only x+wq)
                # PASS B: k heads 0-3 (wk prefetched during A)
                for (wsb, rots) in ((wq_sb, qrot), (wk_sb, krot)):
                    acc_ps = [ps.tile([128, SQT], f32, tag="b",
                                      name=f"qk{st}_{h}")
                              for h in range(HPC)]
                    for d in range(NK):
                        first, last = d == 0, d == NK - 1
                        xt = x_tiles[d]
                        for h in range(HPC):
                            nc.tensor.matmul(acc_ps[h][:],
                                             wsb[d][:, h * HD:(h + 1) * HD],
                                             xt[:], start=first, stop=last)
                    for h in range(HPC):
                        emit_rope(acc_ps[h], rots[h], sq0)
                    if st == 3 and wsb is wq_sb:
                        # wq dead after this strip's A/B passes; prefetch wo
                        # into the freed slots for the output projection
                        for d in range(NK):
                            wot = wqp.tile([128, CW], bf16, tag="wq",
                                           name=f"wo{d}")
                            nc.gpsimd.dma_start(wot[:],
                                                wo[d * 128:(d + 1) * 128, :])
                            wo_sb.append(wot)
                # PASS C: v projection
                v_ps = [ps.tile([128, CW], f32, tag="b", name=f"vps{st}_{ss}")
                        for ss in range(4)]
                for d in range(NK):
                    first, last = d == 0, d == NK - 1
                    for ss in range(4):
                        nc.tensor.matmul(v_ps[ss][:],
                                         x_tiles[d][:, ss * 128:(ss + 1) * 128],
                                         wv_sb[d][:], start=first, stop=last)
                for ss in range(4):
                    nc.scalar.copy(v_sb[st * 4 + ss][:], v_ps[ss][:])

            def emit_attention(sqT):
                sq0 = sqT * SQT
                nblk = 4 * (sqT + 1)
                for h in range(HPC):
                    attn_ps = ps.tile([HD, SQT], f32, tag="b",
                                      name=f"aps{sqT}_{h}")
                    acc = accp.tile([128, SQT], f32, tag="acc",
                                    name=f"acc{sqT}_{h}")
                    exp_tiles = []

                    def emit_pv(j, h=h, attn_ps=attn_ps,
                                exp_tiles=exp_tiles, nblk=nblk):
                        first, last = j == 0, j == nblk - 1
                        e, off = exp_tiles[j]
                        n = SQT - off
                        nc.tensor.matmul(attn_ps[:, off:SQT],
                                         v_sb[j][:, h * HD:(h + 1) * HD],
                                         e[:, 0:n],
                                         start=first, stop=last)

                    for i in range(nblk):
                        r = i - 4 * sqT
                        # diagonal blocks: only sq >= sk is valid; skip the
                        # fully-masked leading columns entirely
                        off = max(0, r) * 128
                        n = SQT - off
                        sc = ps.tile([128, SQT], f32, tag="b",
                                     name=f"sc{sqT}_{h}_{i}")
                        nc.tensor.matmul(sc[:, 0:n],
                                         krot[h][:, i * 128:(i + 1) * 128],
                                         qrot[h][:, sq0 + off:sq0 + SQT],
                                         start=True, stop=True)
                        if r >= 0:  # triangular part within the first strip
                            nc.vector.tensor_add(sc[:, 0:n], sc[:, 0:n],
                                                 mask_sb[r][:, off:SQT])
                        e = expp.tile([128, SQT], bf16, tag="e",
                                      name=f"e{sqT}_{h}_{i}")
                        nc.scalar.activation(e[:, 0:n], sc[:, 0:n],
                                             mybir.ActivationFunctionType.Exp,
                                             scale=SCALE)
                        # denominator: accumulate exp on DVE instead of a
                        # per-block [1,n] PE matmul
                        if i == 0:
                            nc.vector.tensor_copy(acc[:], e[:])
                        else:
                            nc.vector.tensor_add(acc[:, off:SQT],
                                                 acc[:, off:SQT], e[:, 0:n])
                        exp_tiles.append((e, off))
                        if i >= 2:
                            emit_pv(i - 2)
                    emit_pv(nblk - 2)
                    emit_pv(nblk - 1)

                    # acc stays f32 through the accumulation; one bf16
                    # rounding before the ones-reduce averages out over the
                    # 128-partition sum (~0.4%/sqrt(128) den error)
                    acc_bf = accp.tile([128, SQT], bf16, tag="accbf",
                                       bufs=1, name=f"accbf{sqT}_{h}")
                    nc.vector.tensor_copy(acc_bf[:], acc[:])
                    den_ps = ps.tile([1, SQT], f32, tag="b",
                                     name=f"dps{sqT}_{h}")
                    nc.tensor.matmul(den_ps[:], ones_sb[:], acc_bf[:],
                                     start=True, stop=True)
                    # evacuate the attn bank immediately (ACT copy) so the
                    # next head's PSUM allocations never wait on the
                    # recip->broadcast->mul chain (which triggered ~2.2us PE
                    # stalls + HAM K=4/8 re-throttles per head)
                    araw = attnsb.tile([HD, SQT], bf16, tag="a",
                                       name=f"araw{sqT}_{h}")
                    nc.scalar.copy(araw[:], attn_ps[:])
                    rec = nrm.tile([1, SQT], f32, tag="rec",
                                   name=f"rec{sqT}_{h}")
                    nc.vector.reciprocal_approx_fast(out=rec[:], in_=den_ps[:])
                    bc = nrm.tile([128, SQT], f32, tag="bc",
                                  name=f"bc{sqT}_{h}")
                    nc.gpsimd.partition_broadcast(bc[:], rec[:], channels=128)
                    a_sb = attnsb.tile([HD, SQT], bf16, tag="a",
                                       name=f"asb{sqT}_{h}")
                    nc.vector.tensor_mul(a_sb[:], araw[:], bc[:])
                    # store on the gpsimd queue so the sync queue's x/ag loads
                    # are never blocked behind stores
                    nc.gpsimd.dma_start(ag_in[sqT][h * HD:(h + 1) * HD, :],
                                        a_sb[:])
                nc.gpsimd.collective_compute(
                    "AllGather", mybir.AluOpType.bypass, replica_groups=rg,
                    ins=[ag_in[sqT].opt()], outs=[ag_out[sqT].opt()])

            emit_strip(0)
            for st in range(1, NSQ):
                emit_strip(st)
                emit_attention(st - 1)
            emit_attention(NSQ - 1)

            # ================= output projection =================
            for q in range(NSQ):
                o_ps = [ps.tile([128, CW], f32, tag="b", name=f"ops{q}_{ss}")
                        for ss in range(4)]
                for d in range(NK):
                    agt = wkp.tile([128, SQT], bf16, tag="wk",
                                   name=f"agt{q}_{d}")
                    nc.sync.dma_start(agt[:],
                                      ag_out[q][d * 128:(d + 1) * 128, :])
                    first, last = d == 0, d == NK - 1
                    for ss in range(4):
                        nc.tensor.matmul(o_ps[ss][:],
                                         agt[:, ss * 128:(ss + 1) * 128],
                                         wo_sb[d][:], start=first, stop=last)
                for ss in range(4):
                    o = attnsb.tile([128, CW], bf16, tag="a", name=f"o{q}_{ss}")
                    nc.scalar.copy(o[:], o_ps[ss][:])
                    nc.sync.dma_start(
                        out[q * SQT + ss * 128:q * SQT + (ss + 1) * 128, :],
                        o[:])

    nc.compile()
    return nc


def _prep_inputs(x, wq, wk, wv, wo, freqs_cos, freqs_sin, mask):
    bf16 = ml_dtypes.bfloat16
    x2 = np.asarray(x, dtype=np.float32).reshape(S, D)
    xT = np.ascontiguousarray(x2.T).astype(bf16)
    cosT = np.repeat(np.asarray(freqs_cos, np.float32).T, 2, axis=0)
    sinT = np.repeat(np.asarray(freqs_sin, np.float32).T, 2, axis=0).copy()
    sinT[0::2] *= -1.0
    cosT = np.ascontiguousarray(cosT).astype(bf16)
    sinT = np.ascontiguousarray(sinT).astype(bf16)
    m2 = np.asarray(mask, np.float32).reshape(S, S)
    masks = np.stack([np.ascontiguousarray(m2[0:SQT, r * 128:(r + 1) * 128].T)
                      for r in range(4)]).astype(bf16)  # [4, 128, 512]
    in_maps = []
    for c in range(N_CORES):
        cols = slice(c * CW, (c + 1) * CW)
        in_maps.append({
            "xT": xT,
            "wq": np.ascontiguousarray(np.asarray(wq, np.float32)[:, cols]).astype(bf16),
            "wk": np.ascontiguousarray(np.asarray(wk, np.float32)[:, cols]).astype(bf16),
            "wv": np.ascontiguousarray(np.asarray(wv, np.float32)[:, cols]).astype(bf16),
            "wo": np.ascontiguousarray(np.asarray(wo, np.float32)[:, cols]).astype(bf16),
            "cosT": cosT,
            "ones": np.ones((HD, 1), bf16),
            "sinT": sinT,
            "masks": masks,
        })
    return in_maps


def kernel(x, wq, wk, wv, wo, freqs_cos, freqs_sin, mask):
    global LAST_RESULT
    from concourse.bass_utils import run_bass_kernel_spmd

    if "nc" not in _CACHE:
        _CACHE["nc"] = _build()
    nc = _CACHE["nc"]
    in_maps = _prep_inputs(x, wq, wk, wv, wo, freqs_cos, freqs_sin, mask)
    res = run_bass_kernel_spmd(nc, in_maps, core_ids=list(range(N_CORES)))
    LAST_RESULT = res
    out = np.concatenate([res.results[c]["out"].astype(np.float32)
                          for c in range(N_CORES)], axis=1)
    return out.reshape(B, S, D)
